# revision 8
# baseline (speedup 1.0000x reference)
"""GNN message passing on 8 Trainium2 NeuronCores.

Reference computation:
    h = x @ W                       # [N, D]
    msg = h[src]                    # [E, D]
    out = relu(segment_sum(msg, dst, N))

Key identity used here: segment_sum(x[src] @ W) == segment_sum(x[src]) @ W,
so we aggregate raw x rows and apply the small 128x128 matmul once per
output tile at the end.

Distribution: shard edges by DESTINATION range (6272 nodes per core).
Outputs are disjoint -> no all-reduce. Each core:
  1. dma_gather's x rows for its edges (from a replicated node table in
     its local HBM; the table is split into "lo"/"hi" halves so gather
     indices fit in int16, with a zeros row at index 0 for padding),
  2. aggregates them into per-128-dst-window PSUM accumulators via
     TensorE matmuls against one-hot selection matrices built on-device
     with a single DVE compare (iota == dst_local) per 128-edge tile,
  3. applies the final @W matmul + relu per window and DMAs out.

Host-side packing sorts edges into a fixed (window, src-half) slot grid
so all 8 cores run one identical SPMD program.
"""

import numpy as np

P = 128
D = 128
N_NODES = 50000
N_CORES = 8
NW = 49                 # 128-dst windows per core
DPC = NW * P            # dsts per core = 6272
CG = 7                  # windows per gather chunk (NW % CG == 0)
LO_MAX = 32767          # src < LO_MAX -> lo table (idx = src+1 <= 32767)
LO_ROWS = LO_MAX + 1
HI_ROWS = N_NODES - LO_MAX + 1

_NC_CACHE = {}


def _build_nc(t_lo, t_hi, nw=NW, cg=CG, lo_rows=LO_ROWS, hi_rows=HI_ROWS,
              bench_reps=1, parts="all", dma_scratch=16384, n_queues=2,
              bf16=False):
    key = (t_lo, t_hi, nw, cg, lo_rows, hi_rows, bench_reps, parts, dma_scratch,
           n_queues, bf16)
    if key in _NC_CACHE:
        return _NC_CACHE[key]

    import concourse.bacc as bacc
    import concourse.mybir as mybir
    import concourse.tile as tile
    from concourse import library_config

    assert nw % cg == 0
    nch = nw // cg
    n_lo = nw * t_lo * P
    n_hi = nw * t_hi * P

    nc = bacc.Bacc(
        "TRN2", target_bir_lowering=False, debug=False,
        num_swdge_queues=n_queues,
        dynamic_dma_scratch_size=dma_scratch,
    )
    f32 = mybir.dt.float32
    tdt = mybir.dt.bfloat16 if bf16 else f32
    tbl_lo = nc.dram_tensor("tbl_lo", [lo_rows, D], tdt, kind="ExternalInput")
    tbl_hi = nc.dram_tensor("tbl_hi", [hi_rows, D], tdt, kind="ExternalInput")
    idx_lo = nc.dram_tensor(
        "idx_lo", [P, n_lo // 16], mybir.dt.int16, kind="ExternalInput"
    )
    idx_hi = nc.dram_tensor(
        "idx_hi", [P, n_hi // 16], mybir.dt.int16, kind="ExternalInput"
    )
    dloc_lo = nc.dram_tensor("dloc_lo", [P, nw * t_lo], f32, kind="ExternalInput")
    dloc_hi = nc.dram_tensor("dloc_hi", [P, nw * t_hi], f32, kind="ExternalInput")
    wmat = nc.dram_tensor("wmat", [D, D], f32, kind="ExternalInput")
    iota = nc.dram_tensor("iota", [P, P], f32, kind="ExternalInput")
    out = nc.dram_tensor("out", [nw * P, D], f32, kind="ExternalOutput")

    with tile.TileContext(nc) as tc:
        nc.gpsimd.load_library(library_config.mlp)
        with (
            tc.tile_pool(name="const", bufs=1) as cpool,
            tc.tile_pool(name="msga", bufs=2) as apool,
            tc.tile_pool(name="msgb", bufs=2) as bpool,
            tc.tile_pool(name="sel", bufs=8) as spool,
            tc.tile_pool(name="agg", bufs=4) as gpool,
            tc.tile_pool(name="outp", bufs=4) as opool,
            tc.tile_pool(name="psw", bufs=4, space="PSUM") as pwpool,
            tc.tile_pool(name="pso", bufs=2, space="PSUM") as popool,
        ):
            w_sb = cpool.tile([D, D], f32, tag="w")
            nc.sync.dma_start(out=w_sb[:], in_=wmat.ap())
            iota_sb = cpool.tile([P, P], f32, tag="iota")
            nc.sync.dma_start(out=iota_sb[:], in_=iota.ap())
            il_sb = cpool.tile([P, n_lo // 16], mybir.dt.int16, tag="il")
            nc.sync.dma_start(out=il_sb[:], in_=idx_lo.ap())
            ih_sb = cpool.tile([P, n_hi // 16], mybir.dt.int16, tag="ih")
            nc.sync.dma_start(out=ih_sb[:], in_=idx_hi.ap())
            dl_sb = cpool.tile([P, nw * t_lo], f32, tag="dl")
            nc.sync.dma_start(out=dl_sb[:], in_=dloc_lo.ap())
            dh_sb = cpool.tile([P, nw * t_hi], f32, tag="dh")
            nc.sync.dma_start(out=dh_sb[:], in_=dloc_hi.ap())

            def body():
              for ch in range(nch):
                a_tile = apool.tile([P, cg * t_lo, D], tdt, tag="msga")
                b_tile = bpool.tile([P, cg * t_hi, D], tdt, tag="msgb")
                if parts in ("all", "gather"):
                    _emit_gathers(ch, a_tile, b_tile)
                elif parts == "compute":
                    # timing-isolation mode: tiny gather just to mark the
                    # tiles written so the tile allocator accepts the reads
                    nc.gpsimd.dma_gather(
                        a_tile[:, :1, :], tbl_lo.ap(), il_sb[:, :8],
                        P, P, D, queue_num=0, single_packet=False,
                    )
                    nc.gpsimd.dma_gather(
                        b_tile[:, :1, :], tbl_hi.ap(), ih_sb[:, :8],
                        P, P, D, queue_num=1, single_packet=False,
                    )
                if parts in ("all", "compute"):
                    _emit_compute(ch, a_tile, b_tile)

            def _emit_gathers(ch, a_tile, b_tile):
                if n_queues == 2:
                    splits = [(a_tile, tbl_lo, il_sb, t_lo, [(0, cg * t_lo, 0)]),
                              (b_tile, tbl_hi, ih_sb, t_hi, [(0, cg * t_hi, 1)])]
                else:
                    hl = (cg * t_lo + 1) // 2
                    hh = (cg * t_hi + 1) // 2
                    splits = [
                        (a_tile, tbl_lo, il_sb, t_lo,
                         [(0, hl, 0), (hl, cg * t_lo, 1)]),
                        (b_tile, tbl_hi, ih_sb, t_hi,
                         [(0, hh, 2), (hh, cg * t_hi, 3)]),
                    ]
                for tile_, tbl, isb, t, segs in splits:
                    base = ch * cg * t
                    for s0, s1, q in segs:
                        nc.gpsimd.dma_gather(
                            tile_[:, s0:s1, :],
                            tbl.ap(),
                            isb[:, (base + s0) * 8 : (base + s1) * 8],
                            (s1 - s0) * P,
                            (s1 - s0) * P,
                            D,
                            queue_num=q,
                            single_packet=False,
                        )

            def _emit_compute(ch, a_tile, b_tile):
                for wi in range(cg):
                    w = ch * cg + wi
                    psw = pwpool.tile([P, P], f32, tag="psw")
                    nmm = t_lo + t_hi
                    k = 0
                    for t in range(t_lo):
                        sel = spool.tile([P, P], f32, tag="sel")
                        nc.vector.tensor_scalar(
                            sel[:],
                            iota_sb[:],
                            dl_sb[:, w * t_lo + t : w * t_lo + t + 1],
                            None,
                            mybir.AluOpType.is_equal,
                        )
                        nc.tensor.matmul(
                            psw[:],
                            a_tile[:, wi * t_lo + t, :],
                            sel[:],
                            start=(k == 0),
                            stop=(k == nmm - 1),
                        )
                        k += 1
                    for t in range(t_hi):
                        sel = spool.tile([P, P], f32, tag="sel")
                        nc.vector.tensor_scalar(
                            sel[:],
                            iota_sb[:],
                            dh_sb[:, w * t_hi + t : w * t_hi + t + 1],
                            None,
                            mybir.AluOpType.is_equal,
                        )
                        nc.tensor.matmul(
                            psw[:],
                            b_tile[:, wi * t_hi + t, :],
                            sel[:],
                            start=(k == 0),
                            stop=(k == nmm - 1),
                        )
                        k += 1
                    # psw is aggT for this window: [dim, dst_local]
                    agg_t = gpool.tile([P, P], f32, tag="agg")
                    nc.scalar.copy(agg_t[:], psw[:])
                    pso = popool.tile([P, P], f32, tag="pso")
                    nc.tensor.matmul(
                        pso[:], agg_t[:], w_sb[:], start=True, stop=True
                    )
                    o_sb = opool.tile([P, D], f32, tag="out")
                    nc.scalar.activation(
                        o_sb[:], pso[:], mybir.ActivationFunctionType.Relu
                    )
                    nc.sync.dma_start(
                        out=out.ap()[w * P : (w + 1) * P, :], in_=o_sb[:]
                    )

            if bench_reps == 1:
                body()
            else:
                # benchmarking only: repeat the whole body on-device so one
                # PJRT dispatch amortizes its ~90ms overhead over many runs
                with tc.For_i(0, bench_reps, 1):
                    body()

    nc.compile()
    _NC_CACHE[key] = nc
    return nc


def _grid(bucket, mask, order_vals_idx, order_vals_dloc, t, nw=NW, n_cores=N_CORES):
    """Pack one src-half's edges into the fixed per-core slot grid.

    bucket: per-edge (core * nw + window) id, mask: this half's edges.
    Returns idx16 [n_cores, 128, n/16] (int16, wrapped+replicated) and
    dloc [n_cores, 128, nw*t] (f32, -1 for pad slots).
    """
    nb = n_cores * nw
    b = bucket[mask]
    order = np.argsort(b, kind="stable")
    b_sorted = b[order]
    cnts = np.bincount(b_sorted, minlength=nb)
    starts = np.concatenate([[0], np.cumsum(cnts)[:-1]])
    rank = np.arange(len(b_sorted)) - starts[b_sorted]
    spb = t * P  # slots per bucket (window)
    n = nw * spb
    flat_idx = np.zeros((n_cores, n), dtype=np.int16)
    flat_dloc = np.full((n_cores, n), -1.0, dtype=np.float32)
    c = b_sorted // nw
    wloc = b_sorted % nw
    pos = wloc * spb + rank
    flat_idx[c, pos] = order_vals_idx[mask][order]
    flat_dloc[c, pos] = order_vals_dloc[mask][order]
    idx16 = flat_idx.reshape(n_cores, n // 16, 16).transpose(0, 2, 1)
    idx16 = np.ascontiguousarray(np.tile(idx16, (1, 8, 1)))
    dl = np.ascontiguousarray(flat_dloc.reshape(n_cores, nw * t, P).transpose(0, 2, 1))
    return idx16, dl


def _prepare(x, edge_index, W, bf16=False):
    """Host-side packing: returns (t_lo, t_hi, in_maps)."""
    x = np.asarray(x, dtype=np.float32)
    edge_index = np.asarray(edge_index)
    W = np.asarray(W, dtype=np.float32)
    assert x.shape == (N_NODES, D) and W.shape == (D, D)

    src = edge_index[0].astype(np.int64)
    dst = edge_index[1].astype(np.int64)

    core = dst // DPC
    dl_all = dst - core * DPC
    w_all = dl_all >> 7
    dloc_all = (dl_all & 127).astype(np.float32)
    bucket = core * NW + w_all
    is_hi = src >= LO_MAX

    nb = N_CORES * NW
    cnt_lo = np.bincount(bucket[~is_hi], minlength=nb)
    cnt_hi = np.bincount(bucket[is_hi], minlength=nb)
    t_lo = max(1, int(np.ceil(cnt_lo.max() / P)))
    t_hi = max(1, int(np.ceil(cnt_hi.max() / P)))

    idx_val_lo = (src + 1).astype(np.int16, casting="unsafe")
    idx_val_hi = (src - LO_MAX + 1).astype(np.int16, casting="unsafe")
    idx16_lo, dloc_lo = _grid(bucket, ~is_hi, idx_val_lo, dloc_all, t_lo)
    idx16_hi, dloc_hi = _grid(bucket, is_hi, idx_val_hi, dloc_all, t_hi)

    tdt = np.float32
    if bf16:
        import ml_dtypes

        tdt = ml_dtypes.bfloat16
    tbl_lo = np.zeros((LO_ROWS, D), tdt)
    tbl_lo[1:] = x[:LO_MAX].astype(tdt)
    tbl_hi = np.zeros((HI_ROWS, D), tdt)
    tbl_hi[1:] = x[LO_MAX:].astype(tdt)
    iota = np.tile(np.arange(P, dtype=np.float32), (P, 1))
    iota = np.ascontiguousarray(iota)

    in_maps = []
    for c in range(N_CORES):
        in_maps.append(
            {
                "tbl_lo": tbl_lo,
                "tbl_hi": tbl_hi,
                "idx_lo": idx16_lo[c],
                "idx_hi": idx16_hi[c],
                "dloc_lo": dloc_lo[c],
                "dloc_hi": dloc_hi[c],
                "wmat": W,
                "iota": iota,
            }
        )
    return t_lo, t_hi, in_maps


def kernel(x, edge_index, W):
    t_lo, t_hi, in_maps = _prepare(x, edge_index, W)
    nc = _build_nc(t_lo, t_hi)

    from concourse.bass_utils import run_bass_kernel_spmd

    res = run_bass_kernel_spmd(nc, in_maps, core_ids=list(range(N_CORES)))
    # stashed so a test harness can re-run / re-time this invocation
    global _LAST_RUN, _LAST_CAPS
    _LAST_RUN = (nc, in_maps)
    _LAST_CAPS = (t_lo, t_hi)
    outs = [res.results[c]["out"] for c in range(N_CORES)]
    full = np.concatenate(outs, axis=0)[:N_NODES]
    return np.ascontiguousarray(full)


_LAST_RUN = None
_LAST_CAPS = None



# revision 31
# speedup vs baseline: 3.4642x; 3.4642x over previous
"""GNN message passing on 8 Trainium2 NeuronCores.

Reference computation:
    h = x @ W                       # [N, D]
    msg = h[src]                    # [E, D]
    out = relu(segment_sum(msg, dst, N))

Key identity used here: segment_sum(x[src] @ W) == segment_sum(x[src]) @ W,
so we aggregate raw x rows and apply the small 128x128 matmul once per
output tile at the end.

Distribution: shard edges by DESTINATION (6272 output slots per core).
Outputs are disjoint -> no all-reduce. Each core:
  1. dma_gather's bf16 x rows for its edges (replicated bf16 node table in
     local HBM, split into "lo"/"hi" halves so gather indices fit in int16,
     with a zeros row at index 0 for padding), across all 4 SWDGE queues,
  2. aggregates them into per-128-dst-window PSUM accumulators via bf16
     TensorE matmuls against one-hot selection matrices built on-device
     with one batched DVE is_equal (iota == dst_local, broadcast AP) per
     (window, half),
  3. applies the final @W matmul + relu per window and DMAs out.

Host-side packing assigns dst nodes to (core, window, slot) with a greedy
load balancer (minimizes the max per-window lo-edge count, so the padded
tile count per window drops), then sorts edges into the fixed
(window, src-half) slot grid; all 8 cores run one identical SPMD program
and the host inverts the dst permutation on the way out.

Perf notes (measured on HW via on-device For_i repeat loops, device-held
inputs): gather descriptor processing is the bottleneck (~3ns/descriptor
aggregate at 4 queues; descriptor-count-bound, not byte-bound — f32 and
bf16 tables time identically, so bf16 is used to shrink SBUF and skip the
cast). The Tile framework assigns DMASW sem lanes round-robin over the
scheduled order of Pool-DMA instructions; all gathers of a chunk write one
merged tile so the scheduler cannot reorder them, keeping each sem lane
bound to a single SWDGE queue (a cross-queue lane would be a latent race).
"""

import numpy as np

P = 128
D = 128
N_NODES = 50000
N_CORES = 8
NW = 49                 # 128-dst windows per core
DPC = NW * P            # dsts per core = 6272
CG = 7                  # windows per gather chunk (NW % CG == 0)
LO_MAX = 32767          # src < LO_MAX -> lo table (idx = src+1 <= 32767)
LO_ROWS = LO_MAX + 1
HI_ROWS = N_NODES - LO_MAX + 1

_NC_CACHE = {}


def _build_nc(t_lo, t_hi, nw=NW, cg=CG, lo_rows=LO_ROWS, hi_rows=HI_ROWS,
              bench_reps=1, parts="all", dma_scratch=16384, n_queues=2,
              bf16=False, balance=False, single_packet=False, cbf16=False):
    key = (t_lo, t_hi, nw, cg, lo_rows, hi_rows, bench_reps, parts, dma_scratch,
           n_queues, bf16, balance, single_packet, cbf16)
    if key in _NC_CACHE:
        return _NC_CACHE[key]

    import concourse.bacc as bacc
    import concourse.mybir as mybir
    import concourse.tile as tile
    from concourse import library_config

    assert nw % cg == 0
    nch = nw // cg
    n_lo = nw * t_lo * P
    n_hi = nw * t_hi * P

    nc = bacc.Bacc(
        "TRN2", target_bir_lowering=False, debug=False,
        num_swdge_queues=n_queues,
        dynamic_dma_scratch_size=dma_scratch,
    )
    f32 = mybir.dt.float32
    tdt = mybir.dt.bfloat16 if bf16 else f32
    tbl_lo = nc.dram_tensor("tbl_lo", [lo_rows, D], tdt, kind="ExternalInput")
    tbl_hi = nc.dram_tensor("tbl_hi", [hi_rows, D], tdt, kind="ExternalInput")
    idx_lo = nc.dram_tensor(
        "idx_lo", [P, n_lo // 16], mybir.dt.int16, kind="ExternalInput"
    )
    idx_hi = nc.dram_tensor(
        "idx_hi", [P, n_hi // 16], mybir.dt.int16, kind="ExternalInput"
    )
    bf = mybir.dt.bfloat16
    mdt = bf if cbf16 else f32  # metadata dtype (dloc/iota/W)
    sfx = "16" if cbf16 else ""
    dloc_lo = nc.dram_tensor(f"dloc_lo{sfx}", [P, nw * t_lo], mdt, kind="ExternalInput")
    dloc_hi = nc.dram_tensor(f"dloc_hi{sfx}", [P, nw * t_hi], mdt, kind="ExternalInput")
    wmat = nc.dram_tensor(f"wmat{sfx}", [D, D], mdt, kind="ExternalInput")
    iota = nc.dram_tensor(f"iota{sfx}", [P, P], mdt, kind="ExternalInput")
    out = nc.dram_tensor("out", [nw * P, D], f32, kind="ExternalOutput")

    nt_lo = cg * t_lo
    nt = cg * (t_lo + t_hi)

    with tile.TileContext(nc) as tc:
        nc.gpsimd.load_library(library_config.mlp)
        with (
            tc.tile_pool(name="const", bufs=1) as cpool,
            tc.tile_pool(name="msg", bufs=2) as mpool,
            tc.tile_pool(name="m16", bufs=2) as m16pool,
            tc.tile_pool(name="sel", bufs=8) as spool,
            tc.tile_pool(name="agg", bufs=4) as gpool,
            tc.tile_pool(name="outp", bufs=4) as opool,
            tc.tile_pool(name="psw", bufs=4, space="PSUM") as pwpool,
            tc.tile_pool(name="pso", bufs=2, space="PSUM") as popool,
        ):
            w_sb = cpool.tile([D, D], mdt, tag="w")
            nc.sync.dma_start(out=w_sb[:], in_=wmat.ap())
            iota_sb = cpool.tile([P, P], mdt, tag="iota")
            nc.sync.dma_start(out=iota_sb[:], in_=iota.ap())
            il_sb = cpool.tile([P, n_lo // 16], mybir.dt.int16, tag="il")
            nc.sync.dma_start(out=il_sb[:], in_=idx_lo.ap())
            ih_sb = cpool.tile([P, n_hi // 16], mybir.dt.int16, tag="ih")
            nc.sync.dma_start(out=ih_sb[:], in_=idx_hi.ap())
            dl_sb = cpool.tile([P, nw * t_lo], mdt, tag="dl")
            nc.sync.dma_start(out=dl_sb[:], in_=dloc_lo.ap())
            dh_sb = cpool.tile([P, nw * t_hi], mdt, tag="dh")
            nc.sync.dma_start(out=dh_sb[:], in_=dloc_hi.ap())

            def body():
              for ch in range(nch):
                # single merged tile: [lo tiles | hi tiles] so all gathers of
                # a chunk share identical deps (no scheduler reordering,
                # keeping the DMASW lane <-> queue binding consistent)
                m_tile = mpool.tile([P, nt, D], tdt, tag="msg")
                if parts in ("all", "gather"):
                    _emit_gathers(ch, m_tile)
                elif parts == "compute":
                    # timing-isolation mode: tiny gather just to mark the
                    # tile written so the tile allocator accepts the reads
                    nc.gpsimd.dma_gather(
                        m_tile[:, :1, :], tbl_lo.ap(), il_sb[:, :8],
                        P, P, D, queue_num=ch % n_queues,
                        single_packet=False,
                    )
                if parts in ("all", "compute"):
                    if cbf16 and not bf16:
                        m16 = m16pool.tile([P, nt, D], bf, tag="m16")
                        # split the f32->bf16 cast between DVE and ACT
                        csp = (nt * 4) // 10
                        nc.vector.tensor_copy(
                            out=m16[:, :csp, :], in_=m_tile[:, :csp, :]
                        )
                        nc.scalar.copy(m16[:, csp:, :], m_tile[:, csp:, :])
                        _emit_compute_bf16(ch, m16)
                    elif cbf16:
                        _emit_compute_bf16(ch, m_tile)
                    else:
                        _emit_compute(ch, m_tile)

            def _gather_segs(ch):
                """(table_id, tile_start, tile_end, queue) per chunk.

                NOTE: the Tile framework assigns DMASW sem lanes round-robin
                over the SCHEDULED order of Pool-DMA instructions (8 lanes),
                and correctness requires each lane to only ever see one
                queue. All gathers of a chunk write the same tile (identical
                deps -> scheduler keeps emission order); with balance=True
                only the queue NUMBERS swap on odd chunks, a period-2
                pattern that stays lane-consistent under the 8-lane
                round-robin while evening out per-queue descriptor load.
                """
                lo_t, hi_t = cg * t_lo, cg * t_hi
                if n_queues == 2:
                    return [(0, 0, lo_t, 0), (1, 0, hi_t, 1)]
                hl = (lo_t + 1) // 2
                hh = (hi_t + 1) // 2
                if balance and ch % 2 == 1:
                    qs = (2, 3, 0, 1)
                else:
                    qs = (0, 1, 2, 3)
                return [(0, 0, hl, qs[0]), (0, hl, lo_t, qs[1]),
                        (1, 0, hh, qs[2]), (1, hh, hi_t, qs[3])]

            def _emit_gathers(ch, m_tile):
                tbls = [(tbl_lo, il_sb, t_lo, 0), (tbl_hi, ih_sb, t_hi, nt_lo)]
                for tid, s0, s1, q in _gather_segs(ch):
                    tbl, isb, t, off = tbls[tid]
                    base = ch * cg * t
                    nc.gpsimd.dma_gather(
                        m_tile[:, off + s0 : off + s1, :],
                        tbl.ap(),
                        isb[:, (base + s0) * 8 : (base + s1) * 8],
                        (s1 - s0) * P,
                        (s1 - s0) * P,
                        D,
                        queue_num=q,
                        single_packet=single_packet,
                    )

            def _emit_compute(ch, m_tile):
                for wi in range(cg):
                    w = ch * cg + wi
                    psw = pwpool.tile([P, P], f32, tag="psw")
                    nmm = t_lo + t_hi
                    k = 0
                    for t in range(t_lo):
                        sel = spool.tile([P, P], f32, tag="sel")
                        nc.vector.tensor_scalar(
                            sel[:],
                            iota_sb[:],
                            dl_sb[:, w * t_lo + t : w * t_lo + t + 1],
                            None,
                            mybir.AluOpType.is_equal,
                        )
                        nc.tensor.matmul(
                            psw[:],
                            m_tile[:, wi * t_lo + t, :],
                            sel[:],
                            start=(k == 0),
                            stop=(k == nmm - 1),
                        )
                        k += 1
                    for t in range(t_hi):
                        sel = spool.tile([P, P], f32, tag="sel")
                        nc.vector.tensor_scalar(
                            sel[:],
                            iota_sb[:],
                            dh_sb[:, w * t_hi + t : w * t_hi + t + 1],
                            None,
                            mybir.AluOpType.is_equal,
                        )
                        nc.tensor.matmul(
                            psw[:],
                            m_tile[:, nt_lo + wi * t_hi + t, :],
                            sel[:],
                            start=(k == 0),
                            stop=(k == nmm - 1),
                        )
                        k += 1
                    # psw is aggT for this window: [dim, dst_local]
                    agg_t = gpool.tile([P, P], f32, tag="agg")
                    nc.scalar.copy(agg_t[:], psw[:])
                    pso = popool.tile([P, P], f32, tag="pso")
                    nc.tensor.matmul(
                        pso[:], agg_t[:], w_sb[:], start=True, stop=True
                    )
                    o_sb = opool.tile([P, D], f32, tag="out")
                    nc.scalar.activation(
                        o_sb[:], pso[:], mybir.ActivationFunctionType.Relu
                    )
                    nc.sync.dma_start(
                        out=out.ap()[w * P : (w + 1) * P, :], in_=o_sb[:]
                    )

            def _emit_compute_bf16(ch, m16):
                for wi in range(cg):
                    w = ch * cg + wi
                    psw = pwpool.tile([P, P], f32, tag="psw")
                    # batched one-hot builds: one DVE is_equal per half
                    sel_l = spool.tile([P, t_lo, P], bf, tag="sell")
                    nc.vector.tensor_tensor(
                        out=sel_l[:],
                        in0=iota_sb[:].unsqueeze(1).to_broadcast((P, t_lo, P)),
                        in1=dl_sb[:, w * t_lo : (w + 1) * t_lo]
                        .unsqueeze(2)
                        .to_broadcast((P, t_lo, P)),
                        op=mybir.AluOpType.is_equal,
                    )
                    sel_h = spool.tile([P, t_hi, P], bf, tag="selh")
                    nc.vector.tensor_tensor(
                        out=sel_h[:],
                        in0=iota_sb[:].unsqueeze(1).to_broadcast((P, t_hi, P)),
                        in1=dh_sb[:, w * t_hi : (w + 1) * t_hi]
                        .unsqueeze(2)
                        .to_broadcast((P, t_hi, P)),
                        op=mybir.AluOpType.is_equal,
                    )
                    nmm = t_lo + t_hi
                    k = 0
                    for t in range(t_lo):
                        nc.tensor.matmul(
                            psw[:],
                            m16[:, wi * t_lo + t, :],
                            sel_l[:, t, :],
                            start=(k == 0),
                            stop=(k == nmm - 1),
                        )
                        k += 1
                    for t in range(t_hi):
                        nc.tensor.matmul(
                            psw[:],
                            m16[:, nt_lo + wi * t_hi + t, :],
                            sel_h[:, t, :],
                            start=(k == 0),
                            stop=(k == nmm - 1),
                        )
                        k += 1
                    # psw is aggT for this window: [dim, dst_local]
                    agg_t = gpool.tile([P, P], bf, tag="agg16")
                    nc.scalar.copy(agg_t[:], psw[:])
                    pso = popool.tile([P, P], f32, tag="pso")
                    nc.tensor.matmul(
                        pso[:], agg_t[:], w_sb[:], start=True, stop=True
                    )
                    o_sb = opool.tile([P, D], f32, tag="out")
                    nc.scalar.activation(
                        o_sb[:], pso[:], mybir.ActivationFunctionType.Relu
                    )
                    nc.sync.dma_start(
                        out=out.ap()[w * P : (w + 1) * P, :], in_=o_sb[:]
                    )

            if bench_reps == 1:
                body()
            else:
                # benchmarking only: repeat the whole body on-device so one
                # PJRT dispatch amortizes its ~90ms overhead over many runs
                with tc.For_i(0, bench_reps, 1):
                    body()

    nc.compile()
    _NC_CACHE[key] = nc
    return nc


def _grid(bucket, mask, order_vals_idx, order_vals_dloc, t, nw=NW, n_cores=N_CORES):
    """Pack one src-half's edges into the fixed per-core slot grid.

    bucket: per-edge (core * nw + window) id, mask: this half's edges.
    Returns idx16 [n_cores, 128, n/16] (int16, wrapped+replicated) and
    dloc [n_cores, 128, nw*t] (f32, -1 for pad slots).
    """
    nb = n_cores * nw
    b = bucket[mask]
    order = np.argsort(b, kind="stable")
    b_sorted = b[order]
    cnts = np.bincount(b_sorted, minlength=nb)
    starts = np.concatenate([[0], np.cumsum(cnts)[:-1]])
    rank = np.arange(len(b_sorted)) - starts[b_sorted]
    spb = t * P  # slots per bucket (window)
    n = nw * spb
    flat_idx = np.zeros((n_cores, n), dtype=np.int16)
    flat_dloc = np.full((n_cores, n), -1.0, dtype=np.float32)
    c = b_sorted // nw
    wloc = b_sorted % nw
    pos = wloc * spb + rank
    flat_idx[c, pos] = order_vals_idx[mask][order]
    flat_dloc[c, pos] = order_vals_dloc[mask][order]
    idx16 = flat_idx.reshape(n_cores, n // 16, 16).transpose(0, 2, 1)
    idx16 = np.ascontiguousarray(np.tile(idx16, (1, 8, 1)))
    dl = np.ascontiguousarray(flat_dloc.reshape(n_cores, nw * t, P).transpose(0, 2, 1))
    return idx16, dl


def _balance_slots(src, dst):
    """Assign each dst node to a (core, window, dloc) slot, balancing
    per-window lo/hi edge counts so the padded tile counts shrink.

    Returns slot_of_dst [N_NODES] (global slot id = core*DPC + w*128 + dloc).
    """
    is_hi = src >= LO_MAX
    lo_deg = np.bincount(dst[~is_hi], minlength=N_NODES)
    hi_deg = np.bincount(dst[is_hi], minlength=N_NODES)
    slot_of_dst = np.empty(N_NODES, dtype=np.int64)
    for c in range(N_CORES):
        d0, d1 = c * DPC, min((c + 1) * DPC, N_NODES)
        dsts = np.arange(d0, d1)
        order = np.argsort(-lo_deg[dsts], kind="stable")
        lo_sum = np.zeros(NW)
        hi_sum = np.zeros(NW)
        n_in = np.zeros(NW, dtype=np.int64)
        for d in dsts[order]:
            cost = lo_sum + 1e-4 * hi_sum
            cost[n_in >= P] = np.inf
            w = int(np.argmin(cost))
            slot_of_dst[d] = c * DPC + w * P + n_in[w]
            lo_sum[w] += lo_deg[d]
            hi_sum[w] += hi_deg[d]
            n_in[w] += 1
    return slot_of_dst


def _prepare(x, edge_index, W, bf16=False, balance_dsts=True):
    """Host-side packing: returns (t_lo, t_hi, in_maps, slot_of_dst)."""
    x = np.asarray(x, dtype=np.float32)
    edge_index = np.asarray(edge_index)
    W = np.asarray(W, dtype=np.float32)
    assert x.shape == (N_NODES, D) and W.shape == (D, D)

    src = edge_index[0].astype(np.int64)
    dst = edge_index[1].astype(np.int64)

    if balance_dsts:
        slot_of_dst = _balance_slots(src, dst)
        slot = slot_of_dst[dst]
    else:
        slot_of_dst = np.arange(N_NODES, dtype=np.int64)
        slot = dst
    core = slot // DPC
    dl_all = slot - core * DPC
    w_all = dl_all >> 7
    dloc_all = (dl_all & 127).astype(np.float32)
    bucket = core * NW + w_all
    is_hi = src >= LO_MAX

    nb = N_CORES * NW
    cnt_lo = np.bincount(bucket[~is_hi], minlength=nb)
    cnt_hi = np.bincount(bucket[is_hi], minlength=nb)
    t_lo = max(1, int(np.ceil(cnt_lo.max() / P)))
    t_hi = max(1, int(np.ceil(cnt_hi.max() / P)))

    idx_val_lo = (src + 1).astype(np.int16, casting="unsafe")
    idx_val_hi = (src - LO_MAX + 1).astype(np.int16, casting="unsafe")
    idx16_lo, dloc_lo = _grid(bucket, ~is_hi, idx_val_lo, dloc_all, t_lo)
    idx16_hi, dloc_hi = _grid(bucket, is_hi, idx_val_hi, dloc_all, t_hi)

    tdt = np.float32
    if bf16:
        import ml_dtypes

        tdt = ml_dtypes.bfloat16
    tbl_lo = np.zeros((LO_ROWS, D), tdt)
    tbl_lo[1:] = x[:LO_MAX].astype(tdt)
    tbl_hi = np.zeros((HI_ROWS, D), tdt)
    tbl_hi[1:] = x[LO_MAX:].astype(tdt)
    iota = np.tile(np.arange(P, dtype=np.float32), (P, 1))
    iota = np.ascontiguousarray(iota)

    import ml_dtypes

    bf = ml_dtypes.bfloat16
    in_maps = []
    for c in range(N_CORES):
        in_maps.append(
            {
                "tbl_lo": tbl_lo,
                "tbl_hi": tbl_hi,
                "idx_lo": idx16_lo[c],
                "idx_hi": idx16_hi[c],
                "dloc_lo": dloc_lo[c],
                "dloc_hi": dloc_hi[c],
                "wmat": W,
                "iota": iota,
                "dloc_lo16": dloc_lo[c].astype(bf),
                "dloc_hi16": dloc_hi[c].astype(bf),
                "wmat16": W.astype(bf),
                "iota16": iota.astype(bf),
            }
        )
    return t_lo, t_hi, in_maps, slot_of_dst


KERNEL_KW = dict(n_queues=4, balance=True, bf16=True, cbf16=True)


def kernel(x, edge_index, W):
    t_lo, t_hi, in_maps, slot_of_dst = _prepare(
        x, edge_index, W, bf16=KERNEL_KW.get("bf16", False)
    )
    nc = _build_nc(t_lo, t_hi, **KERNEL_KW)

    from concourse.bass_utils import run_bass_kernel_spmd

    res = run_bass_kernel_spmd(nc, in_maps, core_ids=list(range(N_CORES)))
    # stashed so a test harness can re-run / re-time this invocation
    global _LAST_RUN, _LAST_CAPS
    _LAST_RUN = (nc, in_maps)
    _LAST_CAPS = (t_lo, t_hi)
    outs = [res.results[c]["out"] for c in range(N_CORES)]
    full = np.concatenate(outs, axis=0)
    return np.ascontiguousarray(full[slot_of_dst])


_LAST_RUN = None
_LAST_CAPS = None



# revision 34
# speedup vs baseline: 3.8403x; 1.1085x over previous
"""GNN message passing on 8 Trainium2 NeuronCores.

Reference computation:
    h = x @ W                       # [N, D]
    msg = h[src]                    # [E, D]
    out = relu(segment_sum(msg, dst, N))

Key identity used here: segment_sum(x[src] @ W) == segment_sum(x[src]) @ W,
so we aggregate raw x rows and apply the small 128x128 matmul once per
output tile at the end.

Distribution: shard edges by DESTINATION (6272 output slots per core).
Outputs are disjoint -> no all-reduce. Each core:
  1. dma_gather's bf16 x rows for its edges (replicated bf16 node table in
     local HBM, split into "lo"/"hi" halves so gather indices fit in int16,
     with a zeros row at index 0 for padding), across all 4 SWDGE queues,
  2. aggregates them into per-128-dst-window PSUM accumulators via bf16
     TensorE matmuls against one-hot selection matrices built on-device
     with one batched DVE is_equal (iota == dst_local, broadcast AP) per
     (window, half),
  3. applies the final @W matmul + relu per window and DMAs out.

Host-side packing assigns dst nodes to (core, window, slot) with a greedy
load balancer (minimizes the max per-window lo-edge count, so the padded
tile count per window drops), then sorts edges into the fixed
(window, src-half) slot grid; all 8 cores run one identical SPMD program
and the host inverts the dst permutation on the way out.

Perf notes (measured on HW via on-device For_i repeat loops, device-held
inputs): gather descriptor processing is the bottleneck (~3ns/descriptor
aggregate at 4 queues; descriptor-count-bound, not byte-bound — f32 and
bf16 tables time identically, so bf16 is used to shrink SBUF and skip the
cast). The Tile framework assigns DMASW sem lanes round-robin over the
scheduled order of Pool-DMA instructions; all gathers of a chunk write one
merged tile so the scheduler cannot reorder them, keeping each sem lane
bound to a single SWDGE queue (a cross-queue lane would be a latent race).
"""

import numpy as np

P = 128
D = 128
N_NODES = 50000
N_CORES = 8
NW = 49                 # 128-dst windows per core
DPC = NW * P            # dsts per core = 6272
CG = 7                  # windows per gather chunk (NW % CG == 0)
LO_MAX = 32767          # src < LO_MAX -> lo table (idx = src+1 <= 32767)
LO_ROWS = LO_MAX + 1
HI_ROWS = N_NODES - LO_MAX + 1

_NC_CACHE = {}


def _build_nc(t_lo, t_hi, nw=NW, cg=CG, lo_rows=LO_ROWS, hi_rows=HI_ROWS,
              bench_reps=1, parts="all", dma_scratch=16384, n_queues=2,
              bf16=False, balance=False, single_packet=False, cbf16=False,
              mbufs=2):
    key = (t_lo, t_hi, nw, cg, lo_rows, hi_rows, bench_reps, parts, dma_scratch,
           n_queues, bf16, balance, single_packet, cbf16, mbufs)
    if key in _NC_CACHE:
        return _NC_CACHE[key]

    import concourse.bacc as bacc
    import concourse.mybir as mybir
    import concourse.tile as tile
    from concourse import library_config

    assert nw % cg == 0
    nch = nw // cg
    n_lo = nw * t_lo * P
    n_hi = nw * t_hi * P

    nc = bacc.Bacc(
        "TRN2", target_bir_lowering=False, debug=False,
        num_swdge_queues=n_queues,
        dynamic_dma_scratch_size=dma_scratch,
    )
    f32 = mybir.dt.float32
    tdt = mybir.dt.bfloat16 if bf16 else f32
    tbl_lo = nc.dram_tensor("tbl_lo", [lo_rows, D], tdt, kind="ExternalInput")
    tbl_hi = nc.dram_tensor("tbl_hi", [hi_rows, D], tdt, kind="ExternalInput")
    idx_lo = nc.dram_tensor(
        "idx_lo", [P, n_lo // 16], mybir.dt.int16, kind="ExternalInput"
    )
    idx_hi = nc.dram_tensor(
        "idx_hi", [P, n_hi // 16], mybir.dt.int16, kind="ExternalInput"
    )
    bf = mybir.dt.bfloat16
    mdt = bf if cbf16 else f32  # metadata dtype (dloc/iota/W)
    sfx = "16" if cbf16 else ""
    dloc_lo = nc.dram_tensor(f"dloc_lo{sfx}", [P, nw * t_lo], mdt, kind="ExternalInput")
    dloc_hi = nc.dram_tensor(f"dloc_hi{sfx}", [P, nw * t_hi], mdt, kind="ExternalInput")
    wmat = nc.dram_tensor(f"wmat{sfx}", [D, D], mdt, kind="ExternalInput")
    iota = nc.dram_tensor(f"iota{sfx}", [P, P], mdt, kind="ExternalInput")
    out = nc.dram_tensor("out", [nw * P, D], f32, kind="ExternalOutput")

    nt_lo = cg * t_lo
    nt = cg * (t_lo + t_hi)

    with tile.TileContext(nc) as tc:
        nc.gpsimd.load_library(library_config.mlp)
        with (
            tc.tile_pool(name="const", bufs=1) as cpool,
            tc.tile_pool(name="msg", bufs=mbufs) as mpool,
            tc.tile_pool(name="m16", bufs=2) as m16pool,
            tc.tile_pool(name="sel", bufs=8) as spool,
            tc.tile_pool(name="agg", bufs=4) as gpool,
            tc.tile_pool(name="outp", bufs=4) as opool,
            tc.tile_pool(name="psw", bufs=4, space="PSUM") as pwpool,
            tc.tile_pool(name="pso", bufs=2, space="PSUM") as popool,
        ):
            w_sb = cpool.tile([D, D], mdt, tag="w")
            nc.sync.dma_start(out=w_sb[:], in_=wmat.ap())
            iota_sb = cpool.tile([P, P], mdt, tag="iota")
            nc.sync.dma_start(out=iota_sb[:], in_=iota.ap())
            il_sb = cpool.tile([P, n_lo // 16], mybir.dt.int16, tag="il")
            nc.sync.dma_start(out=il_sb[:], in_=idx_lo.ap())
            ih_sb = cpool.tile([P, n_hi // 16], mybir.dt.int16, tag="ih")
            nc.sync.dma_start(out=ih_sb[:], in_=idx_hi.ap())
            dl_sb = cpool.tile([P, nw * t_lo], mdt, tag="dl")
            nc.sync.dma_start(out=dl_sb[:], in_=dloc_lo.ap())
            dh_sb = cpool.tile([P, nw * t_hi], mdt, tag="dh")
            nc.sync.dma_start(out=dh_sb[:], in_=dloc_hi.ap())

            def body():
              for ch in range(nch):
                # single merged tile: [lo tiles | hi tiles] so all gathers of
                # a chunk share identical deps (no scheduler reordering,
                # keeping the DMASW lane <-> queue binding consistent)
                m_tile = mpool.tile([P, nt, D], tdt, tag="msg")
                if parts in ("all", "gather"):
                    _emit_gathers(ch, m_tile)
                elif parts == "compute":
                    # timing-isolation mode: tiny gather just to mark the
                    # tile written so the tile allocator accepts the reads
                    nc.gpsimd.dma_gather(
                        m_tile[:, :1, :], tbl_lo.ap(), il_sb[:, :8],
                        P, P, D, queue_num=ch % n_queues,
                        single_packet=False,
                    )
                if parts in ("all", "compute"):
                    if cbf16 and not bf16:
                        m16 = m16pool.tile([P, nt, D], bf, tag="m16")
                        # split the f32->bf16 cast between DVE and ACT
                        csp = (nt * 4) // 10
                        nc.vector.tensor_copy(
                            out=m16[:, :csp, :], in_=m_tile[:, :csp, :]
                        )
                        nc.scalar.copy(m16[:, csp:, :], m_tile[:, csp:, :])
                        _emit_compute_bf16(ch, m16)
                    elif cbf16:
                        _emit_compute_bf16(ch, m_tile)
                    else:
                        _emit_compute(ch, m_tile)

            def _gather_segs(ch):
                """(table_id, tile_start, tile_end, queue) per chunk.

                NOTE: the Tile framework assigns DMASW sem lanes round-robin
                over the SCHEDULED order of Pool-DMA instructions (8 lanes),
                and correctness requires each lane to only ever see one
                queue. All gathers of a chunk write the same tile (identical
                deps -> scheduler keeps emission order); with balance=True
                only the queue NUMBERS swap on odd chunks, a period-2
                pattern that stays lane-consistent under the 8-lane
                round-robin while evening out per-queue descriptor load.
                """
                lo_t, hi_t = cg * t_lo, cg * t_hi
                if n_queues == 2:
                    return [(0, 0, lo_t, 0), (1, 0, hi_t, 1)]
                hl = (lo_t + 1) // 2
                hh = (hi_t + 1) // 2
                if balance and ch % 2 == 1:
                    qs = (2, 3, 0, 1)
                else:
                    qs = (0, 1, 2, 3)
                return [(0, 0, hl, qs[0]), (0, hl, lo_t, qs[1]),
                        (1, 0, hh, qs[2]), (1, hh, hi_t, qs[3])]

            def _emit_gathers(ch, m_tile):
                tbls = [(tbl_lo, il_sb, t_lo, 0), (tbl_hi, ih_sb, t_hi, nt_lo)]
                for tid, s0, s1, q in _gather_segs(ch):
                    tbl, isb, t, off = tbls[tid]
                    base = ch * cg * t
                    nc.gpsimd.dma_gather(
                        m_tile[:, off + s0 : off + s1, :],
                        tbl.ap(),
                        isb[:, (base + s0) * 8 : (base + s1) * 8],
                        (s1 - s0) * P,
                        (s1 - s0) * P,
                        D,
                        queue_num=q,
                        single_packet=single_packet,
                    )

            def _emit_compute(ch, m_tile):
                for wi in range(cg):
                    w = ch * cg + wi
                    psw = pwpool.tile([P, P], f32, tag="psw")
                    nmm = t_lo + t_hi
                    k = 0
                    for t in range(t_lo):
                        sel = spool.tile([P, P], f32, tag="sel")
                        nc.vector.tensor_scalar(
                            sel[:],
                            iota_sb[:],
                            dl_sb[:, w * t_lo + t : w * t_lo + t + 1],
                            None,
                            mybir.AluOpType.is_equal,
                        )
                        nc.tensor.matmul(
                            psw[:],
                            m_tile[:, wi * t_lo + t, :],
                            sel[:],
                            start=(k == 0),
                            stop=(k == nmm - 1),
                        )
                        k += 1
                    for t in range(t_hi):
                        sel = spool.tile([P, P], f32, tag="sel")
                        nc.vector.tensor_scalar(
                            sel[:],
                            iota_sb[:],
                            dh_sb[:, w * t_hi + t : w * t_hi + t + 1],
                            None,
                            mybir.AluOpType.is_equal,
                        )
                        nc.tensor.matmul(
                            psw[:],
                            m_tile[:, nt_lo + wi * t_hi + t, :],
                            sel[:],
                            start=(k == 0),
                            stop=(k == nmm - 1),
                        )
                        k += 1
                    # psw is aggT for this window: [dim, dst_local]
                    agg_t = gpool.tile([P, P], f32, tag="agg")
                    nc.scalar.copy(agg_t[:], psw[:])
                    pso = popool.tile([P, P], f32, tag="pso")
                    nc.tensor.matmul(
                        pso[:], agg_t[:], w_sb[:], start=True, stop=True
                    )
                    o_sb = opool.tile([P, D], f32, tag="out")
                    nc.scalar.activation(
                        o_sb[:], pso[:], mybir.ActivationFunctionType.Relu
                    )
                    nc.sync.dma_start(
                        out=out.ap()[w * P : (w + 1) * P, :], in_=o_sb[:]
                    )

            def _emit_compute_bf16(ch, m16):
                for wi in range(cg):
                    w = ch * cg + wi
                    psw = pwpool.tile([P, P], f32, tag="psw")
                    # batched one-hot builds: one DVE is_equal per half
                    sel_l = spool.tile([P, t_lo, P], bf, tag="sell")
                    nc.vector.tensor_tensor(
                        out=sel_l[:],
                        in0=iota_sb[:].unsqueeze(1).to_broadcast((P, t_lo, P)),
                        in1=dl_sb[:, w * t_lo : (w + 1) * t_lo]
                        .unsqueeze(2)
                        .to_broadcast((P, t_lo, P)),
                        op=mybir.AluOpType.is_equal,
                    )
                    sel_h = spool.tile([P, t_hi, P], bf, tag="selh")
                    nc.vector.tensor_tensor(
                        out=sel_h[:],
                        in0=iota_sb[:].unsqueeze(1).to_broadcast((P, t_hi, P)),
                        in1=dh_sb[:, w * t_hi : (w + 1) * t_hi]
                        .unsqueeze(2)
                        .to_broadcast((P, t_hi, P)),
                        op=mybir.AluOpType.is_equal,
                    )
                    nmm = t_lo + t_hi
                    k = 0
                    for t in range(t_lo):
                        nc.tensor.matmul(
                            psw[:],
                            m16[:, wi * t_lo + t, :],
                            sel_l[:, t, :],
                            start=(k == 0),
                            stop=(k == nmm - 1),
                        )
                        k += 1
                    for t in range(t_hi):
                        nc.tensor.matmul(
                            psw[:],
                            m16[:, nt_lo + wi * t_hi + t, :],
                            sel_h[:, t, :],
                            start=(k == 0),
                            stop=(k == nmm - 1),
                        )
                        k += 1
                    # psw is aggT for this window: [dim, dst_local]
                    agg_t = gpool.tile([P, P], bf, tag="agg16")
                    nc.scalar.copy(agg_t[:], psw[:])
                    pso = popool.tile([P, P], f32, tag="pso")
                    nc.tensor.matmul(
                        pso[:], agg_t[:], w_sb[:], start=True, stop=True
                    )
                    o_sb = opool.tile([P, D], f32, tag="out")
                    nc.scalar.activation(
                        o_sb[:], pso[:], mybir.ActivationFunctionType.Relu
                    )
                    nc.sync.dma_start(
                        out=out.ap()[w * P : (w + 1) * P, :], in_=o_sb[:]
                    )

            if bench_reps == 1:
                body()
            else:
                # benchmarking only: repeat the whole body on-device so one
                # PJRT dispatch amortizes its ~90ms overhead over many runs
                with tc.For_i(0, bench_reps, 1):
                    body()

    nc.compile()
    _NC_CACHE[key] = nc
    return nc


def _grid(bucket, mask, order_vals_idx, order_vals_dloc, t, nw=NW, n_cores=N_CORES):
    """Pack one src-half's edges into the fixed per-core slot grid.

    bucket: per-edge (core * nw + window) id, mask: this half's edges.
    Returns idx16 [n_cores, 128, n/16] (int16, wrapped+replicated) and
    dloc [n_cores, 128, nw*t] (f32, -1 for pad slots).
    """
    nb = n_cores * nw
    b = bucket[mask]
    order = np.argsort(b, kind="stable")
    b_sorted = b[order]
    cnts = np.bincount(b_sorted, minlength=nb)
    starts = np.concatenate([[0], np.cumsum(cnts)[:-1]])
    rank = np.arange(len(b_sorted)) - starts[b_sorted]
    spb = t * P  # slots per bucket (window)
    n = nw * spb
    flat_idx = np.zeros((n_cores, n), dtype=np.int16)
    flat_dloc = np.full((n_cores, n), -1.0, dtype=np.float32)
    c = b_sorted // nw
    wloc = b_sorted % nw
    pos = wloc * spb + rank
    flat_idx[c, pos] = order_vals_idx[mask][order]
    flat_dloc[c, pos] = order_vals_dloc[mask][order]
    idx16 = flat_idx.reshape(n_cores, n // 16, 16).transpose(0, 2, 1)
    idx16 = np.ascontiguousarray(np.tile(idx16, (1, 8, 1)))
    dl = np.ascontiguousarray(flat_dloc.reshape(n_cores, nw * t, P).transpose(0, 2, 1))
    return idx16, dl


def _balance_slots(src, dst):
    """Assign each dst node to a (core, window, dloc) slot, balancing
    per-window lo/hi edge counts so the padded tile counts shrink.

    Returns slot_of_dst [N_NODES] (global slot id = core*DPC + w*128 + dloc).
    """
    is_hi = src >= LO_MAX
    lo_deg = np.bincount(dst[~is_hi], minlength=N_NODES)
    hi_deg = np.bincount(dst[is_hi], minlength=N_NODES)
    slot_of_dst = np.empty(N_NODES, dtype=np.int64)
    for c in range(N_CORES):
        d0, d1 = c * DPC, min((c + 1) * DPC, N_NODES)
        dsts = np.arange(d0, d1)
        order = np.argsort(-lo_deg[dsts], kind="stable")
        lo_sum = np.zeros(NW)
        hi_sum = np.zeros(NW)
        n_in = np.zeros(NW, dtype=np.int64)
        for d in dsts[order]:
            cost = lo_sum + 1e-4 * hi_sum
            cost[n_in >= P] = np.inf
            w = int(np.argmin(cost))
            slot_of_dst[d] = c * DPC + w * P + n_in[w]
            lo_sum[w] += lo_deg[d]
            hi_sum[w] += hi_deg[d]
            n_in[w] += 1
    return slot_of_dst


def _prepare(x, edge_index, W, bf16=False, balance_dsts=True):
    """Host-side packing: returns (t_lo, t_hi, in_maps, slot_of_dst)."""
    x = np.asarray(x, dtype=np.float32)
    edge_index = np.asarray(edge_index)
    W = np.asarray(W, dtype=np.float32)
    assert x.shape == (N_NODES, D) and W.shape == (D, D)

    src = edge_index[0].astype(np.int64)
    dst = edge_index[1].astype(np.int64)

    if balance_dsts:
        slot_of_dst = _balance_slots(src, dst)
        slot = slot_of_dst[dst]
    else:
        slot_of_dst = np.arange(N_NODES, dtype=np.int64)
        slot = dst
    core = slot // DPC
    dl_all = slot - core * DPC
    w_all = dl_all >> 7
    dloc_all = (dl_all & 127).astype(np.float32)
    bucket = core * NW + w_all
    is_hi = src >= LO_MAX

    nb = N_CORES * NW
    cnt_lo = np.bincount(bucket[~is_hi], minlength=nb)
    cnt_hi = np.bincount(bucket[is_hi], minlength=nb)
    t_lo = max(1, int(np.ceil(cnt_lo.max() / P)))
    t_hi = max(1, int(np.ceil(cnt_hi.max() / P)))

    idx_val_lo = (src + 1).astype(np.int16, casting="unsafe")
    idx_val_hi = (src - LO_MAX + 1).astype(np.int16, casting="unsafe")
    idx16_lo, dloc_lo = _grid(bucket, ~is_hi, idx_val_lo, dloc_all, t_lo)
    idx16_hi, dloc_hi = _grid(bucket, is_hi, idx_val_hi, dloc_all, t_hi)

    tdt = np.float32
    if bf16:
        import ml_dtypes

        tdt = ml_dtypes.bfloat16
    tbl_lo = np.zeros((LO_ROWS, D), tdt)
    tbl_lo[1:] = x[:LO_MAX].astype(tdt)
    tbl_hi = np.zeros((HI_ROWS, D), tdt)
    tbl_hi[1:] = x[LO_MAX:].astype(tdt)
    iota = np.tile(np.arange(P, dtype=np.float32), (P, 1))
    iota = np.ascontiguousarray(iota)

    import ml_dtypes

    bf = ml_dtypes.bfloat16
    in_maps = []
    for c in range(N_CORES):
        in_maps.append(
            {
                "tbl_lo": tbl_lo,
                "tbl_hi": tbl_hi,
                "idx_lo": idx16_lo[c],
                "idx_hi": idx16_hi[c],
                "dloc_lo": dloc_lo[c],
                "dloc_hi": dloc_hi[c],
                "wmat": W,
                "iota": iota,
                "dloc_lo16": dloc_lo[c].astype(bf),
                "dloc_hi16": dloc_hi[c].astype(bf),
                "wmat16": W.astype(bf),
                "iota16": iota.astype(bf),
            }
        )
    return t_lo, t_hi, in_maps, slot_of_dst


KERNEL_KW = dict(n_queues=4, balance=True, bf16=True, cbf16=True, mbufs=4)


def kernel(x, edge_index, W):
    t_lo, t_hi, in_maps, slot_of_dst = _prepare(
        x, edge_index, W, bf16=KERNEL_KW.get("bf16", False)
    )
    nc = _build_nc(t_lo, t_hi, **KERNEL_KW)

    from concourse.bass_utils import run_bass_kernel_spmd

    res = run_bass_kernel_spmd(nc, in_maps, core_ids=list(range(N_CORES)))
    # stashed so a test harness can re-run / re-time this invocation
    global _LAST_RUN, _LAST_CAPS
    _LAST_RUN = (nc, in_maps)
    _LAST_CAPS = (t_lo, t_hi)
    outs = [res.results[c]["out"] for c in range(N_CORES)]
    full = np.concatenate(outs, axis=0)
    return np.ascontiguousarray(full[slot_of_dst])


_LAST_RUN = None
_LAST_CAPS = None

